# revision 24
# baseline (speedup 1.0000x reference)
"""DeBERTa-style 12-layer transformer on 8 TRN2 NeuronCores.

Sharding: data-parallel over batch (B=8 -> 1 sequence per core, no
collectives). Weights are host-packed into per-layer [128, X] fp16 blocks so
each projection loads with 1-6 bulk DMAs per layer (HWDGE descriptor count is
the dominant cost in the baseline). Relative-position tables are expanded on
device via matmul; the (q,k)-dependent gather is a strided "skew" read from a
DRAM scratch buffer, batched to one write + one 3D-AP read per head.
LayerNorms in front of Wout/W2 are folded into post-matmul corrections
(rank-1 mean term + per-token rstd scale).
"""

import math
import numpy as np
import ml_dtypes

import concourse.bacc as bacc
import concourse.bass as bass
import concourse.mybir as mybir
from concourse import tile
from concourse.bass_utils import run_bass_kernel_spmd
from concourse.masks import make_identity

BF = ml_dtypes.bfloat16
F16 = np.float16
bf16 = mybir.dt.bfloat16
fp16 = mybir.dt.float16
f32 = mybir.dt.float32

V = 32768; H = 768; NH = 12; D = 64; L = 12; FI = 2048
S = 512; B = 8; BK = 32; MAXP = 512; EPS = 1e-7
SCALE = 1.0 / math.sqrt(3 * D)
NQT = S // 128      # 4 token tiles
NHT = H // 128      # 6 hidden tiles
WEXP = 640          # per-q-block positional expansion window
CROW = 2 * NQT * WEXP   # 5120: per-partition row in cd scratch (2 tables)
MASK_NEG = -60000.0
AF = mybir.ActivationFunctionType
ALU = mybir.AluOpType


# ---------------------------------------------------------------- host math
def _beta_delta():
    """bucket(delta)+31 for delta in [-511, 511], indexed by delta+511."""
    delta = np.arange(-(S - 1), S)
    sign = np.sign(delta)
    mid = BK // 2
    abs_pos = np.where((delta < mid) & (delta > -mid), mid - 1,
                       np.minimum(np.abs(delta), MAXP - 1))
    log_pos = np.ceil(np.log(abs_pos / mid) / math.log((MAXP - 1) / mid)
                      * (mid - 1)).astype(np.int64) + mid
    bucket = np.where(abs_pos <= mid, delta, log_pos * sign).astype(np.int64)
    return bucket + BK - 1


def _ln_np(x):
    m = x.mean(-1, keepdims=True)
    v = x.var(-1, keepdims=True)
    return (x - m) / np.sqrt(v + EPS)


# ---------------------------------------------------------------- builder
def _build(n_layers, vgb, outb):
    nc = bacc.Bacc("TRN2", target_bir_lowering=False, num_devices=B)

    # ---- dram inputs (host-packed layouts) ----
    wqk = nc.dram_tensor("wqk", [n_layers, 128, 9216], fp16, kind="ExternalInput")
    wvg = nc.dram_tensor("wvg", [n_layers, 128, 9216], fp16, kind="ExternalInput")
    wout = nc.dram_tensor("wout", [n_layers, 128, 4608], fp16, kind="ExternalInput")
    w1 = nc.dram_tensor("w1", [n_layers, 128, 24576], fp16, kind="ExternalInput")
    w2 = nc.dram_tensor("w2", [n_layers, 128, 12288], fp16, kind="ExternalInput")
    tbd = nc.dram_tensor("tbd", [n_layers, NH // 2, 128, 1088], fp16, kind="ExternalInput")
    bqkd = nc.dram_tensor("bqkd", [n_layers, 128, 12], f32, kind="ExternalInput")
    wcsd = nc.dram_tensor("wcsd", [n_layers, 1, 1536], fp16, kind="ExternalInput")
    x0d = nc.dram_tensor("x0d", [NQT, 128, H], f32, kind="ExternalInput")
    maskd = nc.dram_tensor("maskd", [128, NQT], f32, kind="ExternalInput")
    idxd = nc.dram_tensor("idxd", [128, 320], mybir.dt.uint16, kind="ExternalInput")
    yd = nc.dram_tensor("yd", [NQT, 128, H], f32, kind="ExternalOutput")
    if vgb:
        bvgd = nc.dram_tensor("bvgd", [n_layers, 1, 1536], fp16, kind="ExternalInput")
    if outb:
        boutd = nc.dram_tensor("boutd", [n_layers, 1, 1024], fp16, kind="ExternalInput")

    # dram scratch for positional blocks: [par, head, 128, (tbl,qt,j)]
    cd = nc.dram_tensor("cd", [2, NH, 128, CROW], fp16, kind="Internal")

    with tile.TileContext(nc) as tc:
        import contextlib
        ctx = contextlib.ExitStack()
        with ctx:
            pp = ctx.enter_context(tc.tile_pool(name="persist", bufs=1))
            wq = ctx.enter_context(tc.tile_pool(name="wts", bufs=2))
            t5 = ctx.enter_context(tc.tile_pool(name="t512", bufs=1))
            hsp = ctx.enter_context(tc.tile_pool(name="hs768", bufs=1))
            vgc = ctx.enter_context(tc.tile_pool(name="vgc", bufs=1))
            big = ctx.enter_context(tc.tile_pool(name="big", bufs=1))
            sk = ctx.enter_context(tc.tile_pool(name="skew", bufs=2))
            sb = ctx.enter_context(tc.tile_pool(name="work", bufs=2))
            ln = ctx.enter_context(tc.tile_pool(name="lnp", bufs=2))
            ps_mm = ctx.enter_context(tc.tile_pool(name="psmm", bufs=4, space="PSUM"))
            ps_tr = ctx.enter_context(tc.tile_pool(name="pstr", bufs=2, space="PSUM"))
            ps_ctx = ctx.enter_context(tc.tile_pool(name="psctx", bufs=2, space="PSUM"))

            # persistent tiles
            x = [pp.tile([128, H], f32, name=f"x{qt}") for qt in range(NQT)]
            ident = pp.tile([128, 128], fp16, name="ident")
            make_identity(nc, ident[:])
            one_f32 = pp.tile([1, 1], f32, name="one_f32")
            nc.gpsimd.memset(one_f32[:], 1.0)
            ones128 = pp.tile([128, 1], f32, name="ones128")
            nc.gpsimd.memset(ones128[:], 1.0)
            epsb = pp.tile([128, 1], f32, name="epsb")
            nc.gpsimd.memset(epsb[:], EPS)
            maskb = pp.tile([128, NQT], f32, name="maskb")
            nc.sync.dma_start(maskb[:], maskd[:])
            idx1 = pp.tile([128, 320], mybir.dt.uint16, name="idx1")
            nc.sync.dma_start(idx1[:], idxd[:])
            if vgb or outb:
                ones_row = pp.tile([1, 128], fp16, name="ones_row")
                nc.gpsimd.memset(ones_row[:], 1.0)
            for qt in range(NQT):
                nc.sync.dma_start(x[qt][:], x0d[qt, :, :])

            # ---------------- helpers ----------------
            def lstats(chunks, tagp):
                """bn_stats over row chunks -> (mv[128,2]f32, rstd[128,1]f32)."""
                nst = len(chunks)
                stats = ln.tile([128, nst * 6], f32, tag=f"st{nst}")
                for i, cap in enumerate(chunks):
                    nc.vector.bn_stats(stats[:, i * 6:(i + 1) * 6], cap)
                mv = ln.tile([128, 2], f32, tag=f"mv{tagp}", name=f"mv{tagp}", bufs=1)
                nc.vector.bn_aggr(mv[:], stats[:])
                sd = ln.tile([128, 1], f32, tag="sd")
                nc.scalar.activation(sd[:], mv[:, 1:2], AF.Sqrt, bias=epsb[:])
                rstd = ln.tile([128, 1], f32, tag=f"rstd{tagp}", name=f"rstd{tagp}", bufs=1)
                nc.vector.reciprocal(rstd[:], sd[:])
                return mv, rstd

            def chunks_of(t, width):
                if width == H:
                    return [t[:, 0:384], t[:, 384:768]]
                return [t[:, c * 512:(c + 1) * 512] for c in range(width // 512)]

            def ln_one(t, width, out_tile):
                """normalize t -> out_tile (fp16)."""
                mv, rstd = lstats(chunks_of(t, width), "n")
                negb = ln.tile([128, 1], f32, tag="negb")
                nc.vector.scalar_tensor_tensor(
                    negb[:], mv[:, 0:1], -1.0, rstd[:],
                    op0=ALU.mult, op1=ALU.mult)
                nc.scalar.activation(out_tile[:], t[:], AF.Identity,
                                     bias=negb[:], scale=rstd[:])
                return out_tile

            def fold_stats(t, width, tagp):
                """stats for a folded LN: returns (mrow[1,128]fp16 sbuf of -mean,
                rstd[128,1]f32)."""
                mv, rstd = lstats(chunks_of(t, width), tagp)
                mneg = ln.tile([128, 1], fp16, tag=f"mneg{tagp}", name=f"mneg{tagp}", bufs=1)
                nc.vector.scalar_tensor_tensor(
                    mneg[:], mv[:, 0:1], -1.0, ones128[:],
                    op0=ALU.mult, op1=ALU.mult)
                pm = ps_tr.tile([128, 512], fp16, tag="tr")
                nc.tensor.transpose(pm[0:1, 0:128], mneg[:], ident[:])
                mrow = ln.tile([1, 128], fp16, tag=f"mrow{tagp}", name=f"mrow{tagp}", bufs=1)
                nc.vector.tensor_copy(mrow[:], pm[0:1, 0:128])
                return mrow, rstd

            def transpose_h(tiles, n_tiles, tags, width=512):
                """tiles: per-qt [128, n_tiles*128] -> n_tiles x [128, 512]."""
                outs = []
                for hc in range(n_tiles):
                    pt = ps_tr.tile([128, 512], fp16, tag="tr")
                    for qt in range(NQT):
                        nc.tensor.transpose(pt[:, qt * 128:(qt + 1) * 128],
                                            tiles[qt][:, hc * 128:(hc + 1) * 128],
                                            ident[:])
                    o = t5.tile([128, 512], fp16, tag=tags[hc], name=tags[hc])
                    nc.vector.tensor_copy(o[:], pt[:])
                    outs.append(o)
                return outs

            TT = [f"s{i}" for i in range(18)]

            # ---------------- layers ----------------
            hs_next = None
            for li in range(n_layers):
                par = li % 2
                # ---- attention input LN + transpose (LN hoisted into the
                # previous layer's FFN tail for li > 0) ----
                if hs_next is None:
                    hs = []
                    for qt in range(NQT):
                        o = hsp.tile([128, H], fp16, tag=f"h{qt}", name=f"hs{qt}")
                        hs.append(ln_one(x[qt], H, o))
                else:
                    hs = hs_next
                    hs_next = None
                hsT = transpose_h(hs, NHT, TT[12:18])

                bqk_sb = wq.tile([128, 12], f32, tag="bqk")
                nc.sync.dma_start(bqk_sb[:], bqkd[li, :, :])
                wcs_sb = wq.tile([1, 1536], fp16, tag="wcs")
                nc.sync.dma_start(wcs_sb[:], wcsd[li, :, :])

                # ---- QK^T projection: 12 o-tiles [128, 512] (o on partitions) ----
                qkT = []
                for half in range(2):
                    wt = wq.tile([128, 4608], fp16, tag="wproj")
                    nc.sync.dma_start(wt[:], wqk[li, :, half * 4608:(half + 1) * 4608])
                    for oti in range(6):
                        ot = half * 6 + oti
                        po = ps_mm.tile([128, 512], f32, tag="mm")
                        for hc in range(NHT):
                            nc.tensor.matmul(
                                po[:], wt[:, hc * 768 + oti * 128:hc * 768 + (oti + 1) * 128],
                                hsT[hc][:], start=(hc == 0), stop=(hc == NHT - 1))
                        o = t5.tile([128, 512], fp16, tag=TT[ot], name=f"qkT{ot}")
                        sc = SCALE if ot < 6 else 1.0
                        nc.scalar.activation(o[:], po[:], AF.Identity,
                                             bias=bqk_sb[:, ot:ot + 1], scale=sc)
                        qkT.append(o)

                # ---- VG projection: v interleaved with ones col, [tok, head, 65] ----
                v_sb = [vgc.tile([128, NH, D + 1], bf16, tag=f"v{tt}", name=f"v{tt}")
                        for tt in range(NQT)]
                g_sb = [vgc.tile([128, H], fp16, tag=f"g{tt}", name=f"g{tt}")
                        for tt in range(NQT)]
                for tt in range(NQT):
                    nc.gpsimd.memset(v_sb[tt][:, :, D:D + 1], 1.0)
                for oc in range(3):
                    wt = wq.tile([128, 4608], fp16, tag="wproj")
                    nc.sync.dma_start(wt[:, 0:3072], wvg[li, :, oc * 3072:(oc + 1) * 3072])
                    if vgb:
                        bv = wq.tile([1, 512], fp16, tag="bvg")
                        nc.sync.dma_start(bv[:], bvgd[li, :, oc * 512:(oc + 1) * 512])
                    for tt in range(NQT):
                        po = ps_mm.tile([128, 512], f32, tag="mm")
                        for hc in range(NHT):
                            nc.tensor.matmul(po[:], hsT[hc][:, tt * 128:(tt + 1) * 128],
                                             wt[:, hc * 512:(hc + 1) * 512],
                                             start=(hc == 0), stop=(not vgb and hc == NHT - 1))
                        if vgb:
                            nc.tensor.matmul(po[:], ones_row[:], bv[:],
                                             start=False, stop=True)
                        if oc == 0:
                            nc.vector.tensor_copy(v_sb[tt][:, 0:8, 0:D], po[:])
                        elif oc == 1:
                            nc.vector.tensor_copy(v_sb[tt][:, 8:12, 0:D], po[:, 0:256])
                            nc.scalar.copy(g_sb[tt][:, 0:256], po[:, 256:512])
                        else:
                            nc.scalar.copy(g_sb[tt][:, 256:768], po[:])

                # ---- attention per head ----
                # Two-level software pipeline: expansion of head h runs while
                # head h-2's scores consume the DRAM skew round trip, and the
                # per-kt score->rel->exp->ctx chains of head h-2 are woven
                # between head h's expansion matmul pairs so the PE never
                # stalls on the DVE/ACT softmax chain.
                ctx_sb = [vgc.tile([128, H], fp16, tag=f"c{qt}", name=f"ctx{qt}")
                          for qt in range(NQT)]
                tpair = {}
                cw_of = {}
                ctxps_of = {}
                HW = NQT * WEXP

                def qk_of(h):
                    hp = (h % 2) * 64
                    return (qkT[h // 2][hp:hp + 64, :],
                            qkT[6 + h // 2][hp:hp + 64, :], hp)

                def exp_pro(h):
                    if h % 2 == 0:
                        tb_sb = wq.tile([128, 1088], fp16, tag="tb")
                        nc.sync.dma_start(tb_sb[:], tbd[li, h // 2, :, :])
                        tpair[h // 2] = tb_sb
                    cw_of[h] = []

                def exp_sub(h, ti, bt, stg):
                    qT_h, kT_h, hp = qk_of(h)
                    tb_sb = tpair[h // 2]
                    c0 = bt * WEXP
                    if ti == 0:
                        # distinct-bucket projection + gpsimd gather expansion
                        pa = ps_mm.tile([128, 512], f32, tag="mm")
                        nc.tensor.matmul(pa[:, 0:64], qT_h[:, bt * 128:(bt + 1) * 128],
                                         tb_sb[hp:hp + 64, 0:64],
                                         start=True, stop=True)
                        p1 = sb.tile([128, 64], fp16, tag=f"p1{bt % 2}")
                        if bt % 2 == 0:
                            nc.vector.tensor_copy(p1[:], pa[:, 0:64])
                        else:
                            nc.scalar.copy(p1[:], pa[:, 0:64])
                        nc.gpsimd.indirect_copy(
                            stg[:, c0:c0 + WEXP], p1[:],
                            idx1[:, bt * 40:(bt + 1) * 40], True)
                        return
                    j0 = 64 + 384 - 128 * bt
                    pa = ps_mm.tile([128, 512], f32, tag="mm")
                    nc.tensor.matmul(pa[:], kT_h[:, bt * 128:(bt + 1) * 128],
                                     tb_sb[hp:hp + 64, j0:j0 + 512],
                                     start=True, stop=True)
                    pb = ps_mm.tile([128, 512], f32, tag="mm")
                    nc.tensor.matmul(pb[:, 0:128], kT_h[:, bt * 128:(bt + 1) * 128],
                                     tb_sb[hp:hp + 64, j0 + 512:j0 + 640],
                                     start=True, stop=True)
                    if bt % 2 == 0:
                        nc.vector.tensor_copy(stg[:, c0:c0 + 512], pa[:])
                        nc.scalar.copy(stg[:, c0 + 512:c0 + 640], pb[:, 0:128])
                    else:
                        nc.scalar.copy(stg[:, c0:c0 + 512], pa[:])
                        nc.vector.tensor_copy(stg[:, c0 + 512:c0 + 640], pb[:, 0:128])

                def exp_fin(h, ti, stg):
                    base_h = (par * NH + h) * 128 * CROW
                    nc.sync.dma_start(cd[par, h, :, ti * HW:(ti + 1) * HW], stg[:])
                    cw = sk.tile([128, NQT, 512], fp16, tag=f"csk{ti}", bufs=3)
                    nc.sync.dma_start(
                        cw[:], bass.AP(cd, base_h + ti * HW + 127,
                                       [[CROW - 1, 128], [WEXP, NQT], [1, 512]]))
                    cw_of[h].append(cw)

                def att_kt(h, kt):
                    qT_h, kT_h, hp = qk_of(h)
                    cw1, cw2 = cw_of[h]
                    if kt == 0:
                        ctxps_of[h] = ps_ctx.tile([D + 1, 512], f32, tag="ctx", name="ctxps")
                    ctxT_ps = ctxps_of[h]
                    pc2 = ps_tr.tile([128, 512], fp16, tag="tr")
                    for qt in range(NQT):
                        nc.tensor.transpose(pc2[:, qt * 128:(qt + 1) * 128],
                                            cw1[:, qt, kt * 128:(kt + 1) * 128],
                                            ident[:])
                    rel = sb.tile([128, 512], fp16, tag="rel")
                    nc.vector.tensor_add(rel[:], pc2[:], cw2[:, kt, :])
                    ps_s = ps_mm.tile([128, 512], f32, tag="mm")
                    nc.tensor.matmul(ps_s[:], kT_h[:, kt * 128:(kt + 1) * 128],
                                     qT_h[:], start=True, stop=False)
                    nc.tensor.matmul(ps_s[:], ident[:], rel[:],
                                     start=False, stop=True)
                    pT = sb.tile([128, 512], bf16, tag=f"pT{kt}", name=f"pT{kt}")
                    nc.scalar.activation(pT[:], ps_s[:], AF.Exp,
                                         bias=maskb[:, kt:kt + 1])
                    nc.tensor.matmul(ctxT_ps[:], v_sb[kt][:, h, :], pT[:],
                                     start=(kt == 0), stop=(kt == NQT - 1),
                                     skip_group_check=True)

                def att_fin(h):
                    ctxT_ps = ctxps_of.pop(h)
                    cw_of.pop(h)
                    rec = sb.tile([1, 512], f32, tag="rec")
                    nc.vector.reciprocal(rec[:], ctxT_ps[D:D + 1, :])
                    rs_ps = ps_mm.tile([128, 512], f32, tag="mm")
                    for qt in range(NQT):
                        nc.tensor.transpose(rs_ps[:, qt:qt + 1],
                                            rec[:, qt * 128:(qt + 1) * 128],
                                            one_f32[:])
                    rs_sb = sb.tile([128, NQT], f32, tag="rs")
                    nc.vector.tensor_copy(rs_sb[:], rs_ps[:, 0:NQT])
                    ctxT_sb = sb.tile([64, 512], fp16, tag="ctxTsb")
                    nc.scalar.copy(ctxT_sb[:], ctxT_ps[0:D, :])
                    pc = ps_tr.tile([128, 512], fp16, tag="tr")
                    for qt in range(NQT):
                        nc.tensor.transpose(pc[:, qt * 64:(qt + 1) * 64],
                                            ctxT_sb[:, qt * 128:(qt + 1) * 128],
                                            ident[:64, :64])
                    for qt in range(NQT):
                        nc.scalar.activation(ctx_sb[qt][:, h * 64:(h + 1) * 64],
                                             pc[:, qt * 64:(qt + 1) * 64],
                                             AF.Identity, scale=rs_sb[:, qt:qt + 1])

                def stage(he, ha):
                    if he is not None:
                        exp_pro(he)
                    for ti in (0, 1):
                        stg = None
                        if he is not None:
                            stg = sk.tile([128, HW], fp16, tag=f"stg{ti}", name=f"stg{ti}")
                        for bt in range(NQT):
                            if he is not None:
                                exp_sub(he, ti, bt, stg)
                            st = ti * NQT + bt
                            if ha is not None and st % 2 == 1:
                                att_kt(ha, st // 2)
                        if he is not None:
                            exp_fin(he, ti, stg)
                    if ha is not None:
                        att_fin(ha)

                stage(0, None)
                stage(1, None)
                vg_proj()
                stage(2, None)
                for h in range(3, NH):
                    stage(h, h - 3)
                for kt in range(NQT):
                    att_kt(NH - 3, kt)
                    att_kt(NH - 2, kt)
                att_fin(NH - 3)
                att_fin(NH - 2)
                stage(None, NH - 1)

                # ---- gate; Wout LN folded into post-matmul correction ----
                cg = []
                mrow_o, rstd_o = [], []
                for qt in range(NQT):
                    gg = sb.tile([128, H], fp16, tag="gg")
                    nc.scalar.activation(gg[:], g_sb[qt][:], AF.Gelu)
                    t = hsp.tile([128, H], fp16, tag=f"h{qt}", name=f"cg{qt}")
                    nc.vector.tensor_mul(t[:], ctx_sb[qt][:], gg[:])
                    cg.append(t)
                    mr, rs_ = fold_stats(t, H, f"o{qt}")
                    mrow_o.append(mr); rstd_o.append(rs_)
                cgT = transpose_h(cg, NHT, TT[12:18])
                wt = wq.tile([128, 4608], fp16, tag="wproj")
                nc.sync.dma_start(wt[:], wout[li, :, :])
                if outb:
                    bo = wq.tile([1, 1024], fp16, tag="bout")
                    nc.sync.dma_start(bo[:], boutd[li, :, :])
                for qt in range(NQT):
                    for oc in range(2):
                        w = 512 if oc == 0 else H - 512
                        po = ps_mm.tile([128, 512], f32, tag="mm")
                        for hc in range(NHT):
                            nc.tensor.matmul(po[:, :w], cgT[hc][:, qt * 128:(qt + 1) * 128],
                                             wt[:, hc * 768 + oc * 512:hc * 768 + oc * 512 + w],
                                             start=(hc == 0), stop=False)
                        nc.tensor.matmul(po[:, :w], mrow_o[qt][:],
                                         wcs_sb[:, oc * 512:oc * 512 + w],
                                         start=False, stop=True)
                        nc.vector.scalar_tensor_tensor(
                            x[qt][:, oc * 512:oc * 512 + w], po[:, :w],
                            rstd_o[qt][:], x[qt][:, oc * 512:oc * 512 + w],
                            op0=ALU.mult, op1=ALU.add)
                        if outb:
                            pbo = ps_mm.tile([128, 512], f32, tag="mm")
                            nc.tensor.matmul(pbo[:, :w], ones_row[:],
                                             bo[:, oc * 512:oc * 512 + w],
                                             start=True, stop=True)
                            nc.vector.tensor_add(
                                x[qt][:, oc * 512:oc * 512 + w],
                                x[qt][:, oc * 512:oc * 512 + w], pbo[:, :w])

                # ---- FFN (W2 LN folded) ----
                h2 = []
                for qt in range(NQT):
                    o = hsp.tile([128, H], fp16, tag=f"h{qt}", name=f"h2{qt}")
                    h2.append(ln_one(x[qt], H, o))
                h2T = transpose_h(h2, NHT, TT[12:18])
                un = [big.tile([128, FI], fp16, tag=f"un{qt}", name=f"un{qt}")
                      for qt in range(NQT)]
                for c in range(4):
                    wt = wq.tile([128, 6144], fp16, tag="wffn", bufs=3)
                    nc.sync.dma_start(wt[:], w1[li, :, c * 6144:(c + 1) * 6144])
                    for qt in range(NQT):
                        poa = ps_mm.tile([128, 512], f32, tag="mm")
                        for hc in range(NHT):
                            nc.tensor.matmul(poa[:], h2T[hc][:, qt * 128:(qt + 1) * 128],
                                             wt[:, hc * 512:(hc + 1) * 512],
                                             start=(hc == 0), stop=(hc == NHT - 1))
                        pog = ps_mm.tile([128, 512], f32, tag="mm")
                        for hc in range(NHT):
                            nc.tensor.matmul(pog[:], h2T[hc][:, qt * 128:(qt + 1) * 128],
                                             wt[:, 3072 + hc * 512:3072 + (hc + 1) * 512],
                                             start=(hc == 0), stop=(hc == NHT - 1))
                        ffng = sb.tile([128, 512], fp16, tag="ffng")
                        nc.scalar.activation(ffng[:], pog[:], AF.Gelu_apprx_tanh)
                        nc.vector.tensor_mul(un[qt][:, c * 512:(c + 1) * 512],
                                             poa[:], ffng[:])
                mrow_f, rstd_f = [], []
                for qt in range(NQT):
                    mr, rs_ = fold_stats(un[qt], FI, f"f{qt}")
                    mrow_f.append(mr); rstd_f.append(rs_)
                unT = []
                for ic in range(16):
                    pt = ps_tr.tile([128, 512], fp16, tag="tr")
                    for qt in range(NQT):
                        nc.tensor.transpose(pt[:, qt * 128:(qt + 1) * 128],
                                            un[qt][:, ic * 128:(ic + 1) * 128],
                                            ident[:])
                    o = t5.tile([128, 512], fp16, tag=TT[ic], name=f"unT{ic}")
                    nc.vector.tensor_copy(o[:], pt[:])
                    unT.append(o)
                wta = wq.tile([128, 6144], fp16, tag="wffn", bufs=3)
                nc.sync.dma_start(wta[:], w2[li, :, 0:6144])
                wtb = wq.tile([128, 6144], fp16, tag="wffn", bufs=3)
                nc.sync.dma_start(wtb[:], w2[li, :, 6144:12288])
                hs_next = [] if li + 1 < n_layers else None
                for qt in range(NQT):
                    for oc in range(2):
                        w = 512 if oc == 0 else H - 512
                        po = ps_mm.tile([128, 512], f32, tag="mm")
                        for ic in range(16):
                            wt2 = wta if ic < 8 else wtb
                            nc.tensor.matmul(
                                po[:, :w], unT[ic][:, qt * 128:(qt + 1) * 128],
                                wt2[:, (ic % 8) * 768 + oc * 512:(ic % 8) * 768 + oc * 512 + w],
                                start=(ic == 0), stop=False)
                        nc.tensor.matmul(po[:, :w], mrow_f[qt][:],
                                         wcs_sb[:, 768 + oc * 512:768 + oc * 512 + w],
                                         start=False, stop=True)
                        nc.vector.scalar_tensor_tensor(
                            x[qt][:, oc * 512:oc * 512 + w], po[:, :w],
                            rstd_f[qt][:], x[qt][:, oc * 512:oc * 512 + w],
                            op0=ALU.mult, op1=ALU.add)
                    if hs_next is not None:
                        o = hsp.tile([128, H], fp16, tag=f"h{qt}", name=f"hsn{qt}")
                        hs_next.append(ln_one(x[qt], H, o))

            # ---- output ----
            for qt in range(NQT):
                nc.sync.dma_start(yd[qt, :, :], x[qt][:])

    nc.finalize()
    return nc


_CACHE = {}


def _get_nc(n_layers, vgb=False, outb=False):
    key = (n_layers, vgb, outb)
    if key not in _CACHE:
        _CACHE[key] = _build(n_layers, vgb, outb)
    return _CACHE[key]


# ---------------------------------------------------------------- host prep
def _prep_shared(word_emb, rel_emb, rel_g, rel_b, Wqk, bqk, Wvg, bvg, Wout,
                 bout, W1, W2, n_layers):
    beta = _beta_delta()                     # [1023]
    idx_c2p = beta[1022 - np.arange(1023)]   # T1: delta = 511 - j
    idx_p2c = beta[np.arange(1023)]          # T2: delta = j - 511
    rel = _ln_np(rel_emb.astype(np.float64)).astype(np.float32) * rel_g + rel_b

    d = {}
    tb = np.zeros((n_layers, NH, 64, 1088), np.float32)
    wqk_t = np.zeros((n_layers, 128, 9216), np.float32)
    wvg_t = np.zeros((n_layers, 128, 9216), np.float32)
    wout_t = np.zeros((n_layers, 128, 4608), np.float32)
    w1_t = np.zeros((n_layers, 128, 24576), np.float32)
    w2_t = np.zeros((n_layers, 128, 12288), np.float32)
    bqk_t = np.zeros((n_layers, 128, 12), np.float32)
    wcs_t = np.zeros((n_layers, 1, 1536), np.float32)
    for li in range(n_layers):
        pos = rel @ Wqk[li].T + bqk[li]          # [63, 1536]
        qpos = pos[:, :H].reshape(63, NH, 64)
        kpos = pos[:, H:].reshape(63, NH, 64)
        tb[li, :, :, :63] = kpos.transpose(1, 2, 0)
        tb[li, :, :, 64:1087] = qpos[idx_p2c].transpose(1, 2, 0) * SCALE

        # wqk: [p, half, hc, oti, j]
        a = Wqk[li].T.reshape(NHT, 128, 2, 6, 128)       # [hc, p, half, oti, j]
        wqk_t[li] = a.transpose(1, 2, 0, 3, 4).reshape(128, 9216)
        # wvg: [p, oc, hc, j]
        a = Wvg[li].T.reshape(NHT, 128, 3, 512)
        wvg_t[li] = a.transpose(1, 2, 0, 3).reshape(128, 9216)
        # wout: [p, hc, o768]
        a = Wout[li].T.reshape(NHT, 128, H)
        wout_t[li] = a.transpose(1, 0, 2).reshape(128, 4608)
        # w1: [p, pair-chunk c, role r, hc, j]; oc = c + 4r
        a = W1[li].T.reshape(NHT, 128, 8, 512).transpose(1, 2, 0, 3)  # [p, oc, hc, j]
        w1_t[li] = a[:, [0, 4, 1, 5, 2, 6, 3, 7]].reshape(128, 24576)
        # w2: [p, d, ici, o768]
        a = W2[li].T.reshape(2, 8, 128, H).transpose(2, 0, 1, 3)
        w2_t[li] = a.reshape(128, 12288)
        bqk_t[li] = bqk[li].reshape(12, 128).T
        wcs_t[li, 0, :H] = Wout[li].sum(axis=1)
        wcs_t[li, 0, H:] = W2[li].sum(axis=1)

    d["wqk"] = wqk_t.astype(F16)
    d["wvg"] = wvg_t.astype(F16)
    d["wout"] = wout_t.astype(F16)
    d["w1"] = w1_t.astype(F16)
    d["w2"] = w2_t.astype(F16)
    d["tbd"] = tb.reshape(n_layers, NH // 2, 128, 1088).astype(F16)
    idx_full = np.concatenate([idx_c2p, [63]]).astype(np.uint16)  # pad -> zero col
    idxv = np.zeros((128, 320), np.uint16)
    for bt in range(NQT):
        j0 = 384 - 128 * bt
        for p in range(128):
            for s_ in range(40):
                idxv[p, bt * 40 + s_] = idx_full[j0 + s_ * 16 + (p % 16)]
    d["idxd"] = idxv
    d["bqkd"] = bqk_t
    d["wcsd"] = wcs_t.astype(F16)
    vgb = bool(np.any(bvg))
    outb = bool(np.any(bout))
    if vgb:
        d["bvgd"] = bvg.reshape(n_layers, 1, 1536).astype(F16)
    if outb:
        bo = np.zeros((n_layers, 1, 1024), np.float32)
        bo[:, 0, :H] = bout
        d["boutd"] = bo.astype(F16)
    return d, vgb, outb


def _make_in_maps(inputs, n_layers):
    input_ids = np.asarray(inputs["input_ids"])
    attention_mask = np.asarray(inputs["attention_mask"])
    word_emb = np.asarray(inputs["word_emb"], np.float32)

    shared, vgb, outb = _prep_shared(
        word_emb, np.asarray(inputs["rel_emb"], np.float32),
        np.asarray(inputs["rel_g"], np.float32), np.asarray(inputs["rel_b"], np.float32),
        np.asarray(inputs["Wqk"], np.float32), np.asarray(inputs["bqk"], np.float32),
        np.asarray(inputs["Wvg"], np.float32), np.asarray(inputs["bvg"], np.float32),
        np.asarray(inputs["Wout"], np.float32), np.asarray(inputs["bout"], np.float32),
        np.asarray(inputs["W1"], np.float32), np.asarray(inputs["W2"], np.float32),
        n_layers)

    in_maps = []
    for b in range(B):
        m = dict(shared)
        x0 = _ln_np(word_emb[input_ids[:, b]].astype(np.float64)).astype(np.float32)
        m["x0d"] = x0.reshape(NQT, 128, H)
        mb = np.where(attention_mask[b, 0, 0, :], MASK_NEG, 0.0).astype(np.float32)
        m["maskd"] = mb.reshape(NQT, 128).T.copy()
        in_maps.append(m)
    return in_maps, vgb, outb


def run(inputs, n_layers=L, trace=False):
    in_maps, vgb, outb = _make_in_maps(inputs, n_layers)
    nc = _get_nc(n_layers, vgb, outb)
    res = run_bass_kernel_spmd(nc, in_maps, core_ids=list(range(B)), trace=trace)
    out = np.zeros((S, B, H), np.float32)
    for b in range(B):
        out[:, b, :] = res.results[b]["yd"].reshape(S, H)
    return out, res


def kernel(**inputs) -> np.ndarray:
    out, _ = run(inputs, L)
    return out


# ------------------------------------------------------- timing-only runner
def make_timed_runner(n_layers, in_maps, nc=None):
    """Build a persistent jitted PJRT callable over 8 cores for wall-clock
    timing (the axon NTFF profile hook is unavailable in this container)."""
    import jax
    from jax.sharding import Mesh, PartitionSpec, NamedSharding
    from jax.experimental.shard_map import shard_map
    from concourse import bass2jax

    if nc is None:
        nc = _get_nc(n_layers)
    bass2jax.install_neuronx_cc_hook()
    partition_name = nc.partition_id_tensor.name if nc.partition_id_tensor else None
    in_names, out_names, out_avals, zero_outs = [], [], [], []
    import concourse.mybir as _mb
    for alloc in nc.m.functions[0].allocations:
        if not isinstance(alloc, _mb.MemoryLocationSet):
            continue
        name = alloc.memorylocations[0].name
        if alloc.kind == "ExternalInput":
            if name != partition_name:
                in_names.append(name)
        elif alloc.kind == "ExternalOutput":
            out_names.append(name)
            shape = tuple(alloc.tensor_shape)
            dtype = _mb.dt.np(alloc.dtype)
            out_avals.append(jax.core.ShapedArray(shape, dtype))
            zero_outs.append(np.zeros(shape, dtype))
    n_params = len(in_names)
    n_outs = len(out_avals)
    all_in_names = list(in_names) + out_names
    if partition_name is not None:
        all_in_names = all_in_names + [partition_name]

    def _body(*args):
        operands = list(args)
        if partition_name is not None:
            operands.append(bass2jax.partition_id_tensor())
        outs = bass2jax._bass_exec_p.bind(
            *operands, out_avals=tuple(out_avals), in_names=tuple(all_in_names),
            out_names=tuple(out_names), lowering_input_output_aliases=(),
            sim_require_finite=True, sim_require_nnan=True, nc=nc)
        return tuple(outs)

    n_cores = B
    devices = jax.devices()[:n_cores]
    mesh = Mesh(np.asarray(devices), ("core",))
    P = PartitionSpec
    sharded = jax.jit(
        shard_map(_body, mesh=mesh, in_specs=(P("core"),) * (n_params + n_outs),
                  out_specs=(P("core"),) * n_outs, check_rep=False),
        keep_unused=True)

    concat_in = [
        np.concatenate([np.asarray(in_maps[c][nm]) for c in range(n_cores)], axis=0)
        for nm in in_names]
    concat_zeros = [np.zeros((n_cores * z.shape[0], *z.shape[1:]), z.dtype)
                    for z in zero_outs]
    shard = NamedSharding(mesh, P("core"))
    dev_in = [jax.device_put(a, shard) for a in concat_in]
    dev_zeros = [jax.device_put(a, shard) for a in concat_zeros]

    def call():
        outs = sharded(*dev_in, *dev_zeros)
        jax.block_until_ready(outs)
        return outs

    return call


# revision 28
# speedup vs baseline: 1.1527x; 1.1527x over previous
"""DeBERTa-style 12-layer transformer on 8 TRN2 NeuronCores.

Sharding: data-parallel over batch (B=8 -> 1 sequence per core, no
collectives). Weights are host-packed into per-layer [128, X] fp16 blocks so
each projection loads with 1-6 bulk DMAs per layer (HWDGE descriptor count is
the dominant cost in the baseline). Relative-position tables are expanded on
device via matmul; the (q,k)-dependent gather is a strided "skew" read from a
DRAM scratch buffer, batched to one write + one 3D-AP read per head.
LayerNorms in front of Wout/W2 are folded into post-matmul corrections
(rank-1 mean term + per-token rstd scale).
"""

import math
import numpy as np
import ml_dtypes

import concourse.bacc as bacc
import concourse.bass as bass
import concourse.mybir as mybir
from concourse import tile
from concourse.bass_utils import run_bass_kernel_spmd
from concourse.masks import make_identity

BF = ml_dtypes.bfloat16
F16 = np.float16
bf16 = mybir.dt.bfloat16
fp16 = mybir.dt.float16
f32 = mybir.dt.float32

V = 32768; H = 768; NH = 12; D = 64; L = 12; FI = 2048
S = 512; B = 8; BK = 32; MAXP = 512; EPS = 1e-7
SCALE = 1.0 / math.sqrt(3 * D)
NQT = S // 128      # 4 token tiles
NHT = H // 128      # 6 hidden tiles
WEXP = 640          # per-q-block positional expansion window
CROW = 2 * NQT * WEXP   # 5120: per-partition row in cd scratch (2 tables)
MASK_NEG = -60000.0
AF = mybir.ActivationFunctionType
ALU = mybir.AluOpType


# ---------------------------------------------------------------- host math
def _beta_delta():
    """bucket(delta)+31 for delta in [-511, 511], indexed by delta+511."""
    delta = np.arange(-(S - 1), S)
    sign = np.sign(delta)
    mid = BK // 2
    abs_pos = np.where((delta < mid) & (delta > -mid), mid - 1,
                       np.minimum(np.abs(delta), MAXP - 1))
    log_pos = np.ceil(np.log(abs_pos / mid) / math.log((MAXP - 1) / mid)
                      * (mid - 1)).astype(np.int64) + mid
    bucket = np.where(abs_pos <= mid, delta, log_pos * sign).astype(np.int64)
    return bucket + BK - 1


def _ln_np(x):
    m = x.mean(-1, keepdims=True)
    v = x.var(-1, keepdims=True)
    return (x - m) / np.sqrt(v + EPS)


# ---------------------------------------------------------------- builder
def _build(n_layers, vgb, outb):
    nc = bacc.Bacc("TRN2", target_bir_lowering=False, num_devices=B)

    # ---- dram inputs (host-packed layouts) ----
    wqk = nc.dram_tensor("wqk", [n_layers, 128, 9216], fp16, kind="ExternalInput")
    wvg = nc.dram_tensor("wvg", [n_layers, 128, 9216], fp16, kind="ExternalInput")
    wout = nc.dram_tensor("wout", [n_layers, 128, 4608], fp16, kind="ExternalInput")
    w1 = nc.dram_tensor("w1", [n_layers, 128, 24576], fp16, kind="ExternalInput")
    w2 = nc.dram_tensor("w2", [n_layers, 128, 12288], fp16, kind="ExternalInput")
    tbd = nc.dram_tensor("tbd", [n_layers, NH // 2, 128, 1088], fp16, kind="ExternalInput")
    bqkd = nc.dram_tensor("bqkd", [n_layers, 128, 12], f32, kind="ExternalInput")
    wcsd = nc.dram_tensor("wcsd", [n_layers, 1, 1536], fp16, kind="ExternalInput")
    x0d = nc.dram_tensor("x0d", [NQT, 128, H], f32, kind="ExternalInput")
    maskd = nc.dram_tensor("maskd", [128, NQT], f32, kind="ExternalInput")
    idxd = nc.dram_tensor("idxd", [128, 320], mybir.dt.uint16, kind="ExternalInput")
    yd = nc.dram_tensor("yd", [NQT, 128, H], f32, kind="ExternalOutput")
    if vgb:
        bvgd = nc.dram_tensor("bvgd", [n_layers, 1, 1536], fp16, kind="ExternalInput")
    if outb:
        boutd = nc.dram_tensor("boutd", [n_layers, 1, 1024], fp16, kind="ExternalInput")

    # dram scratch for positional blocks: [par, head, 128, (tbl,qt,j)]
    cd = nc.dram_tensor("cd", [2, NH, 128, CROW], fp16, kind="Internal")

    with tile.TileContext(nc) as tc:
        import contextlib
        ctx = contextlib.ExitStack()
        with ctx:
            pp = ctx.enter_context(tc.tile_pool(name="persist", bufs=1))
            wq = ctx.enter_context(tc.tile_pool(name="wts", bufs=2))
            t5 = ctx.enter_context(tc.tile_pool(name="t512", bufs=1))
            hsp = ctx.enter_context(tc.tile_pool(name="hs768", bufs=1))
            vgc = ctx.enter_context(tc.tile_pool(name="vgc", bufs=1))
            big = ctx.enter_context(tc.tile_pool(name="big", bufs=1))
            sk = ctx.enter_context(tc.tile_pool(name="skew", bufs=2))
            sb = ctx.enter_context(tc.tile_pool(name="work", bufs=2))
            ln = ctx.enter_context(tc.tile_pool(name="lnp", bufs=2))
            ps_mm = ctx.enter_context(tc.tile_pool(name="psmm", bufs=4, space="PSUM"))
            ps_tr = ctx.enter_context(tc.tile_pool(name="pstr", bufs=2, space="PSUM"))
            ps_ctx = ctx.enter_context(tc.tile_pool(name="psctx", bufs=2, space="PSUM"))

            # persistent tiles
            x = [pp.tile([128, H], f32, name=f"x{qt}") for qt in range(NQT)]
            ident = pp.tile([128, 128], fp16, name="ident")
            make_identity(nc, ident[:])
            one_f32 = pp.tile([1, 1], f32, name="one_f32")
            nc.gpsimd.memset(one_f32[:], 1.0)
            ones128 = pp.tile([128, 1], f32, name="ones128")
            nc.gpsimd.memset(ones128[:], 1.0)
            epsb = pp.tile([128, 1], f32, name="epsb")
            nc.gpsimd.memset(epsb[:], EPS)
            maskb = pp.tile([128, NQT], f32, name="maskb")
            nc.sync.dma_start(maskb[:], maskd[:])
            idx1 = pp.tile([128, 320], mybir.dt.uint16, name="idx1")
            nc.sync.dma_start(idx1[:], idxd[:])
            if vgb or outb:
                ones_row = pp.tile([1, 128], fp16, name="ones_row")
                nc.gpsimd.memset(ones_row[:], 1.0)
            for qt in range(NQT):
                nc.sync.dma_start(x[qt][:], x0d[qt, :, :])

            # ---------------- helpers ----------------
            def lstats(chunks, tagp):
                """bn_stats over row chunks -> (mv[128,2]f32, rstd[128,1]f32)."""
                nst = len(chunks)
                stats = ln.tile([128, nst * 6], f32, tag=f"st{nst}")
                for i, cap in enumerate(chunks):
                    nc.vector.bn_stats(stats[:, i * 6:(i + 1) * 6], cap)
                mv = ln.tile([128, 2], f32, tag=f"mv{tagp}", name=f"mv{tagp}", bufs=1)
                nc.vector.bn_aggr(mv[:], stats[:])
                sd = ln.tile([128, 1], f32, tag="sd")
                nc.scalar.activation(sd[:], mv[:, 1:2], AF.Sqrt, bias=epsb[:])
                rstd = ln.tile([128, 1], f32, tag=f"rstd{tagp}", name=f"rstd{tagp}", bufs=1)
                nc.vector.reciprocal(rstd[:], sd[:])
                return mv, rstd

            def chunks_of(t, width):
                if width == H:
                    return [t[:, 0:384], t[:, 384:768]]
                return [t[:, c * 512:(c + 1) * 512] for c in range(width // 512)]

            def ln_one(t, width, out_tile):
                """normalize t -> out_tile (fp16)."""
                mv, rstd = lstats(chunks_of(t, width), "n")
                negb = ln.tile([128, 1], f32, tag="negb")
                nc.vector.scalar_tensor_tensor(
                    negb[:], mv[:, 0:1], -1.0, rstd[:],
                    op0=ALU.mult, op1=ALU.mult)
                nc.scalar.activation(out_tile[:], t[:], AF.Identity,
                                     bias=negb[:], scale=rstd[:])
                return out_tile

            def fold_stats(t, width, tagp):
                """stats for a folded LN: returns (mrow[1,128]fp16 sbuf of -mean,
                rstd[128,1]f32)."""
                mv, rstd = lstats(chunks_of(t, width), tagp)
                mneg = ln.tile([128, 1], fp16, tag=f"mneg{tagp}", name=f"mneg{tagp}", bufs=1)
                nc.vector.scalar_tensor_tensor(
                    mneg[:], mv[:, 0:1], -1.0, ones128[:],
                    op0=ALU.mult, op1=ALU.mult)
                pm = ps_tr.tile([128, 512], fp16, tag="tr")
                nc.tensor.transpose(pm[0:1, 0:128], mneg[:], ident[:])
                mrow = ln.tile([1, 128], fp16, tag=f"mrow{tagp}", name=f"mrow{tagp}", bufs=1)
                nc.vector.tensor_copy(mrow[:], pm[0:1, 0:128])
                return mrow, rstd

            def transpose_h(tiles, n_tiles, tags, width=512):
                """tiles: per-qt [128, n_tiles*128] -> n_tiles x [128, 512]."""
                outs = []
                for hc in range(n_tiles):
                    pt = ps_tr.tile([128, 512], fp16, tag="tr")
                    for qt in range(NQT):
                        nc.tensor.transpose(pt[:, qt * 128:(qt + 1) * 128],
                                            tiles[qt][:, hc * 128:(hc + 1) * 128],
                                            ident[:])
                    o = t5.tile([128, 512], fp16, tag=tags[hc], name=tags[hc])
                    nc.vector.tensor_copy(o[:], pt[:])
                    outs.append(o)
                return outs

            TT = [f"s{i}" for i in range(18)]

            # ---------------- layers ----------------
            hs_next = None
            for li in range(n_layers):
                par = li % 2
                # ---- attention input LN + transpose (LN hoisted into the
                # previous layer's FFN tail for li > 0) ----
                if hs_next is None:
                    hs = []
                    for qt in range(NQT):
                        o = hsp.tile([128, H], fp16, tag=f"h{qt}", name=f"hs{qt}")
                        hs.append(ln_one(x[qt], H, o))
                else:
                    hs = hs_next
                    hs_next = None
                hsT = transpose_h(hs, NHT, TT[12:18])

                bqk_sb = wq.tile([128, 12], f32, tag="bqk")
                nc.sync.dma_start(bqk_sb[:], bqkd[li, :, :])
                wcs_sb = wq.tile([1, 1536], fp16, tag="wcs")
                nc.sync.dma_start(wcs_sb[:], wcsd[li, :, :])

                # ---- QK^T projection: 12 o-tiles [128, 512] (o on partitions) ----
                qkT = []
                for half in range(2):
                    wt = wq.tile([128, 4608], fp16, tag="wproj")
                    nc.sync.dma_start(wt[:], wqk[li, :, half * 4608:(half + 1) * 4608])
                    for oti in range(6):
                        ot = half * 6 + oti
                        po = ps_mm.tile([128, 512], f32, tag="mm")
                        for hc in range(NHT):
                            nc.tensor.matmul(
                                po[:], wt[:, hc * 768 + oti * 128:hc * 768 + (oti + 1) * 128],
                                hsT[hc][:], start=(hc == 0), stop=(hc == NHT - 1))
                        o = t5.tile([128, 512], fp16, tag=TT[ot], name=f"qkT{ot}")
                        sc = SCALE if ot < 6 else 1.0
                        nc.scalar.activation(o[:], po[:], AF.Identity,
                                             bias=bqk_sb[:, ot:ot + 1], scale=sc)
                        qkT.append(o)

                # ---- VG projection: v interleaved with ones col, [tok, head, 65] ----
                v_sb = [vgc.tile([128, NH, D + 1], bf16, tag=f"v{tt}", name=f"v{tt}")
                        for tt in range(NQT)]
                g_sb = [vgc.tile([128, H], fp16, tag=f"g{tt}", name=f"g{tt}")
                        for tt in range(NQT)]
                for tt in range(NQT):
                    nc.gpsimd.memset(v_sb[tt][:, :, D:D + 1], 1.0)
                for oc in range(3):
                    wt = wq.tile([128, 4608], fp16, tag="wproj")
                    nc.sync.dma_start(wt[:, 0:3072], wvg[li, :, oc * 3072:(oc + 1) * 3072])
                    if vgb:
                        bv = wq.tile([1, 512], fp16, tag="bvg")
                        nc.sync.dma_start(bv[:], bvgd[li, :, oc * 512:(oc + 1) * 512])
                    for tt in range(NQT):
                        po = ps_mm.tile([128, 512], f32, tag="mm")
                        for hc in range(NHT):
                            nc.tensor.matmul(po[:], hsT[hc][:, tt * 128:(tt + 1) * 128],
                                             wt[:, hc * 512:(hc + 1) * 512],
                                             start=(hc == 0), stop=(not vgb and hc == NHT - 1))
                        if vgb:
                            nc.tensor.matmul(po[:], ones_row[:], bv[:],
                                             start=False, stop=True)
                        if oc == 0:
                            nc.vector.tensor_copy(v_sb[tt][:, 0:8, 0:D], po[:])
                        elif oc == 1:
                            nc.vector.tensor_copy(v_sb[tt][:, 8:12, 0:D], po[:, 0:256])
                            nc.scalar.copy(g_sb[tt][:, 0:256], po[:, 256:512])
                        else:
                            nc.scalar.copy(g_sb[tt][:, 256:768], po[:])

                # ---- attention per head ----
                # Two-level software pipeline: expansion of head h runs while
                # head h-2's scores consume the DRAM skew round trip, and the
                # per-kt score->rel->exp->ctx chains of head h-2 are woven
                # between head h's expansion matmul pairs so the PE never
                # stalls on the DVE/ACT softmax chain.
                ctx_sb = [vgc.tile([128, H], fp16, tag=f"c{qt}", name=f"ctx{qt}")
                          for qt in range(NQT)]
                tpair = {}
                cw_of = {}
                ctxps_of = {}
                HW = NQT * WEXP

                def qk_of(h):
                    hp = (h % 2) * 64
                    return (qkT[h // 2][hp:hp + 64, :],
                            qkT[6 + h // 2][hp:hp + 64, :], hp)

                def exp_pro(h):
                    if h % 2 == 0:
                        tb_sb = wq.tile([128, 1088], fp16, tag="tb")
                        nc.sync.dma_start(tb_sb[:], tbd[li, h // 2, :, :])
                        tpair[h // 2] = tb_sb
                    cw_of[h] = []

                def exp_sub(h, ti, bt, stg):
                    qT_h, kT_h, hp = qk_of(h)
                    tb_sb = tpair[h // 2]
                    c0 = bt * WEXP
                    if ti == 0:
                        # distinct-bucket projection + gpsimd gather expansion
                        pa = ps_mm.tile([128, 512], f32, tag="mm")
                        nc.tensor.matmul(pa[:, 0:64], qT_h[:, bt * 128:(bt + 1) * 128],
                                         tb_sb[hp:hp + 64, 0:64],
                                         start=True, stop=True)
                        p1 = sb.tile([128, 64], fp16, tag=f"p1{bt % 2}")
                        if bt % 2 == 0:
                            nc.vector.tensor_copy(p1[:], pa[:, 0:64])
                        else:
                            nc.scalar.copy(p1[:], pa[:, 0:64])
                        nc.gpsimd.indirect_copy(
                            stg[:, c0:c0 + WEXP], p1[:],
                            idx1[:, bt * 40:(bt + 1) * 40], True)
                        return
                    j0 = 64 + 384 - 128 * bt
                    pa = ps_mm.tile([128, 512], f32, tag="mm")
                    nc.tensor.matmul(pa[:], kT_h[:, bt * 128:(bt + 1) * 128],
                                     tb_sb[hp:hp + 64, j0:j0 + 512],
                                     start=True, stop=True)
                    pb = ps_mm.tile([128, 512], f32, tag="mm")
                    nc.tensor.matmul(pb[:, 0:128], kT_h[:, bt * 128:(bt + 1) * 128],
                                     tb_sb[hp:hp + 64, j0 + 512:j0 + 640],
                                     start=True, stop=True)
                    if bt % 2 == 0:
                        nc.vector.tensor_copy(stg[:, c0:c0 + 512], pa[:])
                        nc.scalar.copy(stg[:, c0 + 512:c0 + 640], pb[:, 0:128])
                    else:
                        nc.scalar.copy(stg[:, c0:c0 + 512], pa[:])
                        nc.vector.tensor_copy(stg[:, c0 + 512:c0 + 640], pb[:, 0:128])

                def exp_fin(h, ti, stg):
                    base_h = (par * NH + h) * 128 * CROW
                    nc.sync.dma_start(cd[par, h, :, ti * HW:(ti + 1) * HW], stg[:])
                    cw = sk.tile([128, NQT, 512], fp16, tag=f"csk{ti}", bufs=3)
                    nc.sync.dma_start(
                        cw[:], bass.AP(cd, base_h + ti * HW + 127,
                                       [[CROW - 1, 128], [WEXP, NQT], [1, 512]]))
                    cw_of[h].append(cw)

                def att_kt(h, kt):
                    qT_h, kT_h, hp = qk_of(h)
                    cw1, cw2 = cw_of[h]
                    if kt == 0:
                        ctxps_of[h] = ps_ctx.tile([D + 1, 512], f32, tag="ctx", name="ctxps")
                    ctxT_ps = ctxps_of[h]
                    pc2 = ps_tr.tile([128, 512], fp16, tag="tr")
                    for qt in range(NQT):
                        nc.tensor.transpose(pc2[:, qt * 128:(qt + 1) * 128],
                                            cw1[:, qt, kt * 128:(kt + 1) * 128],
                                            ident[:])
                    rel = sb.tile([128, 512], fp16, tag="rel")
                    nc.vector.tensor_add(rel[:], pc2[:], cw2[:, kt, :])
                    ps_s = ps_mm.tile([128, 512], f32, tag="mm")
                    nc.tensor.matmul(ps_s[:], kT_h[:, kt * 128:(kt + 1) * 128],
                                     qT_h[:], start=True, stop=False)
                    nc.tensor.matmul(ps_s[:], ident[:], rel[:],
                                     start=False, stop=True)
                    pT = sb.tile([128, 512], bf16, tag=f"pT{kt}", name=f"pT{kt}")
                    nc.scalar.activation(pT[:], ps_s[:], AF.Exp,
                                         bias=maskb[:, kt:kt + 1])
                    nc.tensor.matmul(ctxT_ps[:], v_sb[kt][:, h, :], pT[:],
                                     start=(kt == 0), stop=(kt == NQT - 1),
                                     skip_group_check=True)

                def att_fin(h):
                    ctxT_ps = ctxps_of.pop(h)
                    cw_of.pop(h)
                    rec = sb.tile([1, 512], f32, tag="rec")
                    nc.vector.reciprocal(rec[:], ctxT_ps[D:D + 1, :])
                    rs_ps = ps_mm.tile([128, 512], f32, tag="mm")
                    for qt in range(NQT):
                        nc.tensor.transpose(rs_ps[:, qt:qt + 1],
                                            rec[:, qt * 128:(qt + 1) * 128],
                                            one_f32[:])
                    rs_sb = sb.tile([128, NQT], f32, tag="rs")
                    nc.vector.tensor_copy(rs_sb[:], rs_ps[:, 0:NQT])
                    ctxT_sb = sb.tile([64, 512], fp16, tag="ctxTsb")
                    nc.scalar.copy(ctxT_sb[:], ctxT_ps[0:D, :])
                    pc = ps_tr.tile([128, 512], fp16, tag="tr")
                    for qt in range(NQT):
                        nc.tensor.transpose(pc[:, qt * 64:(qt + 1) * 64],
                                            ctxT_sb[:, qt * 128:(qt + 1) * 128],
                                            ident[:64, :64])
                    for qt in range(NQT):
                        nc.scalar.activation(ctx_sb[qt][:, h * 64:(h + 1) * 64],
                                             pc[:, qt * 64:(qt + 1) * 64],
                                             AF.Identity, scale=rs_sb[:, qt:qt + 1])

                def stage(he, ha):
                    if he is not None:
                        exp_pro(he)
                    for ti in (0, 1):
                        stg = None
                        if he is not None:
                            stg = sk.tile([128, HW], fp16, tag=f"stg{ti}", name=f"stg{ti}")
                        for bt in range(NQT):
                            if he is not None:
                                exp_sub(he, ti, bt, stg)
                            st = ti * NQT + bt
                            if ha is not None and st % 2 == 1:
                                att_kt(ha, st // 2)
                        if he is not None:
                            exp_fin(he, ti, stg)
                    if ha is not None:
                        att_fin(ha)

                stage(0, None)
                stage(1, None)
                vg_proj()
                stage(2, None)
                for h in range(3, NH):
                    stage(h, h - 3)
                for kt in range(NQT):
                    att_kt(NH - 3, kt)
                    att_kt(NH - 2, kt)
                att_fin(NH - 3)
                att_fin(NH - 2)
                stage(None, NH - 1)

                # ---- gate; Wout LN folded into post-matmul correction ----
                cg = []
                mrow_o, rstd_o = [], []
                for qt in range(NQT):
                    gg = sb.tile([128, H], fp16, tag="gg")
                    nc.scalar.activation(gg[:], g_sb[qt][:], AF.Gelu)
                    t = hsp.tile([128, H], fp16, tag=f"h{qt}", name=f"cg{qt}")
                    nc.vector.tensor_mul(t[:], ctx_sb[qt][:], gg[:])
                    cg.append(t)
                    mr, rs_ = fold_stats(t, H, f"o{qt}")
                    mrow_o.append(mr); rstd_o.append(rs_)
                cgT = transpose_h(cg, NHT, TT[12:18])
                wt = wq.tile([128, 4608], fp16, tag="wproj")
                nc.sync.dma_start(wt[:], wout[li, :, :])
                if outb:
                    bo = wq.tile([1, 1024], fp16, tag="bout")
                    nc.sync.dma_start(bo[:], boutd[li, :, :])
                for qt in range(NQT):
                    for oc in range(2):
                        w = 512 if oc == 0 else H - 512
                        po = ps_mm.tile([128, 512], f32, tag="mm")
                        for hc in range(NHT):
                            nc.tensor.matmul(po[:, :w], cgT[hc][:, qt * 128:(qt + 1) * 128],
                                             wt[:, hc * 768 + oc * 512:hc * 768 + oc * 512 + w],
                                             start=(hc == 0), stop=False)
                        nc.tensor.matmul(po[:, :w], mrow_o[qt][:],
                                         wcs_sb[:, oc * 512:oc * 512 + w],
                                         start=False, stop=True)
                        nc.vector.scalar_tensor_tensor(
                            x[qt][:, oc * 512:oc * 512 + w], po[:, :w],
                            rstd_o[qt][:], x[qt][:, oc * 512:oc * 512 + w],
                            op0=ALU.mult, op1=ALU.add)
                        if outb:
                            pbo = ps_mm.tile([128, 512], f32, tag="mm")
                            nc.tensor.matmul(pbo[:, :w], ones_row[:],
                                             bo[:, oc * 512:oc * 512 + w],
                                             start=True, stop=True)
                            nc.vector.tensor_add(
                                x[qt][:, oc * 512:oc * 512 + w],
                                x[qt][:, oc * 512:oc * 512 + w], pbo[:, :w])

                # ---- FFN (W2 LN folded) ----
                h2 = []
                for qt in range(NQT):
                    o = hsp.tile([128, H], fp16, tag=f"h{qt}", name=f"h2{qt}")
                    h2.append(ln_one(x[qt], H, o))
                h2T = transpose_h(h2, NHT, TT[12:18])
                un = [big.tile([128, FI], fp16, tag=f"un{qt}", name=f"un{qt}")
                      for qt in range(NQT)]
                for c in range(4):
                    wt = wq.tile([128, 6144], fp16, tag="wffn", bufs=3)
                    nc.sync.dma_start(wt[:], w1[li, :, c * 6144:(c + 1) * 6144])
                    for qt in range(NQT):
                        poa = ps_mm.tile([128, 512], f32, tag="mm")
                        for hc in range(NHT):
                            nc.tensor.matmul(poa[:], h2T[hc][:, qt * 128:(qt + 1) * 128],
                                             wt[:, hc * 512:(hc + 1) * 512],
                                             start=(hc == 0), stop=(hc == NHT - 1))
                        pog = ps_mm.tile([128, 512], f32, tag="mm")
                        for hc in range(NHT):
                            nc.tensor.matmul(pog[:], h2T[hc][:, qt * 128:(qt + 1) * 128],
                                             wt[:, 3072 + hc * 512:3072 + (hc + 1) * 512],
                                             start=(hc == 0), stop=(hc == NHT - 1))
                        ffng = sb.tile([128, 512], fp16, tag="ffng")
                        nc.scalar.activation(ffng[:], pog[:], AF.Gelu_apprx_tanh)
                        nc.vector.tensor_mul(un[qt][:, c * 512:(c + 1) * 512],
                                             poa[:], ffng[:])
                mrow_f, rstd_f = [], []
                for qt in range(NQT):
                    mr, rs_ = fold_stats(un[qt], FI, f"f{qt}")
                    mrow_f.append(mr); rstd_f.append(rs_)
                unT = []
                for ic in range(16):
                    pt = ps_tr.tile([128, 512], fp16, tag="tr")
                    for qt in range(NQT):
                        nc.tensor.transpose(pt[:, qt * 128:(qt + 1) * 128],
                                            un[qt][:, ic * 128:(ic + 1) * 128],
                                            ident[:])
                    o = t5.tile([128, 512], fp16, tag=TT[ic], name=f"unT{ic}")
                    nc.vector.tensor_copy(o[:], pt[:])
                    unT.append(o)
                wta = wq.tile([128, 6144], fp16, tag="wffn", bufs=3)
                nc.sync.dma_start(wta[:], w2[li, :, 0:6144])
                wtb = wq.tile([128, 6144], fp16, tag="wffn", bufs=3)
                nc.sync.dma_start(wtb[:], w2[li, :, 6144:12288])
                hs_next = [] if li + 1 < n_layers else None
                for qt in range(NQT):
                    for oc in range(2):
                        w = 512 if oc == 0 else H - 512
                        po = ps_mm.tile([128, 512], f32, tag="mm")
                        for ic in range(16):
                            wt2 = wta if ic < 8 else wtb
                            nc.tensor.matmul(
                                po[:, :w], unT[ic][:, qt * 128:(qt + 1) * 128],
                                wt2[:, (ic % 8) * 768 + oc * 512:(ic % 8) * 768 + oc * 512 + w],
                                start=(ic == 0), stop=False)
                        nc.tensor.matmul(po[:, :w], mrow_f[qt][:],
                                         wcs_sb[:, 768 + oc * 512:768 + oc * 512 + w],
                                         start=False, stop=True)
                        nc.vector.scalar_tensor_tensor(
                            x[qt][:, oc * 512:oc * 512 + w], po[:, :w],
                            rstd_f[qt][:], x[qt][:, oc * 512:oc * 512 + w],
                            op0=ALU.mult, op1=ALU.add)
                    if hs_next is not None:
                        o = hsp.tile([128, H], fp16, tag=f"h{qt}", name=f"hsn{qt}")
                        hs_next.append(ln_one(x[qt], H, o))

            # ---- output ----
            for qt in range(NQT):
                nc.sync.dma_start(yd[qt, :, :], x[qt][:])

    nc.finalize()
    return nc


_CACHE = {}


def _get_nc(n_layers, vgb=False, outb=False):
    key = (n_layers, vgb, outb)
    if key not in _CACHE:
        _CACHE[key] = _build(n_layers, vgb, outb)
    return _CACHE[key]


# ---------------------------------------------------------------- host prep
def _prep_shared(word_emb, rel_emb, rel_g, rel_b, Wqk, bqk, Wvg, bvg, Wout,
                 bout, W1, W2, n_layers):
    beta = _beta_delta()                     # [1023]
    idx_c2p = beta[1022 - np.arange(1023)]   # T1: delta = 511 - j
    idx_p2c = beta[np.arange(1023)]          # T2: delta = j - 511
    rel = _ln_np(rel_emb.astype(np.float64)).astype(np.float32) * rel_g + rel_b

    d = {}
    tb = np.zeros((n_layers, NH, 64, 1088), np.float32)
    wqk_t = np.zeros((n_layers, 128, 9216), np.float32)
    wvg_t = np.zeros((n_layers, 128, 9216), np.float32)
    wout_t = np.zeros((n_layers, 128, 4608), np.float32)
    w1_t = np.zeros((n_layers, 128, 24576), np.float32)
    w2_t = np.zeros((n_layers, 128, 12288), np.float32)
    bqk_t = np.zeros((n_layers, 128, 12), np.float32)
    wcs_t = np.zeros((n_layers, 1, 1536), np.float32)
    for li in range(n_layers):
        pos = rel @ Wqk[li].T + bqk[li]          # [63, 1536]
        qpos = pos[:, :H].reshape(63, NH, 64)
        kpos = pos[:, H:].reshape(63, NH, 64)
        tb[li, :, :, :63] = kpos.transpose(1, 2, 0)
        tb[li, :, :, 64:1087] = qpos[idx_p2c].transpose(1, 2, 0) * SCALE

        # wqk: [p, half, hc, oti, j]
        a = Wqk[li].T.reshape(NHT, 128, 2, 6, 128)       # [hc, p, half, oti, j]
        wqk_t[li] = a.transpose(1, 2, 0, 3, 4).reshape(128, 9216)
        # wvg: [p, oc, hc, j]
        a = Wvg[li].T.reshape(NHT, 128, 3, 512)
        wvg_t[li] = a.transpose(1, 2, 0, 3).reshape(128, 9216)
        # wout: [p, hc, o768]
        a = Wout[li].T.reshape(NHT, 128, H)
        wout_t[li] = a.transpose(1, 0, 2).reshape(128, 4608)
        # w1: [p, pair-chunk c, role r, hc, j]; oc = c + 4r
        a = W1[li].T.reshape(NHT, 128, 8, 512).transpose(1, 2, 0, 3)  # [p, oc, hc, j]
        w1_t[li] = a[:, [0, 4, 1, 5, 2, 6, 3, 7]].reshape(128, 24576)
        # w2: [p, d, ici, o768]
        a = W2[li].T.reshape(2, 8, 128, H).transpose(2, 0, 1, 3)
        w2_t[li] = a.reshape(128, 12288)
        bqk_t[li] = bqk[li].reshape(12, 128).T
        wcs_t[li, 0, :H] = Wout[li].sum(axis=1)
        wcs_t[li, 0, H:] = W2[li].sum(axis=1)

    d["wqk"] = wqk_t.astype(F16)
    d["wvg"] = wvg_t.astype(F16)
    d["wout"] = wout_t.astype(F16)
    d["w1"] = w1_t.astype(F16)
    d["w2"] = w2_t.astype(F16)
    d["tbd"] = tb.reshape(n_layers, NH // 2, 128, 1088).astype(F16)
    idx_full = np.concatenate([idx_c2p, [63]]).astype(np.uint16)  # pad -> zero col
    idxv = np.zeros((128, 320), np.uint16)
    for bt in range(NQT):
        j0 = 384 - 128 * bt
        for p in range(128):
            for s_ in range(40):
                idxv[p, bt * 40 + s_] = idx_full[j0 + s_ * 16 + (p % 16)]
    d["idxd"] = idxv
    d["bqkd"] = bqk_t
    d["wcsd"] = wcs_t.astype(F16)
    vgb = bool(np.any(bvg))
    outb = bool(np.any(bout))
    if vgb:
        d["bvgd"] = bvg.reshape(n_layers, 1, 1536).astype(F16)
    if outb:
        bo = np.zeros((n_layers, 1, 1024), np.float32)
        bo[:, 0, :H] = bout
        d["boutd"] = bo.astype(F16)
    return d, vgb, outb


def _make_in_maps(inputs, n_layers):
    input_ids = np.asarray(inputs["input_ids"])
    attention_mask = np.asarray(inputs["attention_mask"])
    word_emb = np.asarray(inputs["word_emb"], np.float32)

    shared, vgb, outb = _prep_shared(
        word_emb, np.asarray(inputs["rel_emb"], np.float32),
        np.asarray(inputs["rel_g"], np.float32), np.asarray(inputs["rel_b"], np.float32),
        np.asarray(inputs["Wqk"], np.float32), np.asarray(inputs["bqk"], np.float32),
        np.asarray(inputs["Wvg"], np.float32), np.asarray(inputs["bvg"], np.float32),
        np.asarray(inputs["Wout"], np.float32), np.asarray(inputs["bout"], np.float32),
        np.asarray(inputs["W1"], np.float32), np.asarray(inputs["W2"], np.float32),
        n_layers)

    in_maps = []
    for b in range(B):
        m = dict(shared)
        x0 = _ln_np(word_emb[input_ids[:, b]].astype(np.float64)).astype(np.float32)
        m["x0d"] = x0.reshape(NQT, 128, H)
        mb = np.where(attention_mask[b, 0, 0, :], MASK_NEG, 0.0).astype(np.float32)
        m["maskd"] = mb.reshape(NQT, 128).T.copy()
        in_maps.append(m)
    return in_maps, vgb, outb


def run(inputs, n_layers=L, trace=False):
    in_maps, vgb, outb = _make_in_maps(inputs, n_layers)
    nc = _get_nc(n_layers, vgb, outb)
    res = run_bass_kernel_spmd(nc, in_maps, core_ids=list(range(B)), trace=trace)
    out = np.zeros((S, B, H), np.float32)
    for b in range(B):
        out[:, b, :] = res.results[b]["yd"].reshape(S, H)
    return out, res


def kernel(**inputs) -> np.ndarray:
    out, _ = run(inputs, L)
    return out


# ------------------------------------------------------- timing-only runner
def make_timed_runner(n_layers, in_maps, nc=None):
    """Build a persistent jitted PJRT callable over 8 cores for wall-clock
    timing (the axon NTFF profile hook is unavailable in this container)."""
    import jax
    from jax.sharding import Mesh, PartitionSpec, NamedSharding
    from jax.experimental.shard_map import shard_map
    from concourse import bass2jax

    if nc is None:
        nc = _get_nc(n_layers)
    bass2jax.install_neuronx_cc_hook()
    partition_name = nc.partition_id_tensor.name if nc.partition_id_tensor else None
    in_names, out_names, out_avals, zero_outs = [], [], [], []
    import concourse.mybir as _mb
    for alloc in nc.m.functions[0].allocations:
        if not isinstance(alloc, _mb.MemoryLocationSet):
            continue
        name = alloc.memorylocations[0].name
        if alloc.kind == "ExternalInput":
            if name != partition_name:
                in_names.append(name)
        elif alloc.kind == "ExternalOutput":
            out_names.append(name)
            shape = tuple(alloc.tensor_shape)
            dtype = _mb.dt.np(alloc.dtype)
            out_avals.append(jax.core.ShapedArray(shape, dtype))
            zero_outs.append(np.zeros(shape, dtype))
    n_params = len(in_names)
    n_outs = len(out_avals)
    all_in_names = list(in_names) + out_names
    if partition_name is not None:
        all_in_names = all_in_names + [partition_name]

    def _body(*args):
        operands = list(args)
        if partition_name is not None:
            operands.append(bass2jax.partition_id_tensor())
        outs = bass2jax._bass_exec_p.bind(
            *operands, out_avals=tuple(out_avals), in_names=tuple(all_in_names),
            out_names=tuple(out_names), lowering_input_output_aliases=(),
            sim_require_finite=True, sim_require_nnan=True, nc=nc)
        return tuple(outs)

    n_cores = B
    devices = jax.devices()[:n_cores]
    mesh = Mesh(np.asarray(devices), ("core",))
    P = PartitionSpec
    sharded = jax.jit(
        shard_map(_body, mesh=mesh, in_specs=(P("core"),) * (n_params + n_outs),
                  out_specs=(P("core"),) * n_outs, check_rep=False),
        keep_unused=True)

    concat_in = [
        np.concatenate([np.asarray(in_maps[c][nm]) for c in range(n_cores)], axis=0)
        for nm in in_names]
    concat_zeros = [np.zeros((n_cores * z.shape[0], *z.shape[1:]), z.dtype)
                    for z in zero_outs]
    shard = NamedSharding(mesh, P("core"))
    dev_in = [jax.device_put(a, shard) for a in concat_in]
    dev_zeros = [jax.device_put(a, shard) for a in concat_zeros]

    def call():
        outs = sharded(*dev_in, *dev_zeros)
        jax.block_until_ready(outs)
        return outs

    return call
